# revision 23
# baseline (speedup 1.0000x reference)
"""Self-contained Trainium2 Bass kernel for MultiHeadAttention with QK-layernorm
and physical-coordinate RoPE.

Sharding: 8 cores = 4 batches x 2 head-groups (8 heads each).  Each core
computes its batch's projections for its head group, attention, and a partial
output projection (row-sharded Wo); the host sums the two partials per batch.
"""

import math
import sys
import types

import numpy as np
import ml_dtypes

# ---- problem constants (hardcoded; kernel.py must not read spec/reference) ----
B, S, DM = 4, 2048, 1536
H_TOT, DH = 16, 96
HG = 8                      # heads per core
DV = HG * DH                # 768 per-core projection width
PHYS, NF = 3, 16            # phys dims, freqs
MIN_LF, MAX_LF = -5.0, 3.0
LN_EPS = 1e-5
N_CORES = 8

SQ_TILES = S // 128         # 16
K_TILES = DM // 128         # 12
PROJ_CHUNK = 384            # 4 heads worth of dv per psum chunk
SCALE = 1.0 / math.sqrt(DH)

# Cody-Waite 3-term split of 2*pi (c1/c2 have trailing mantissa zeroed so
# k*c1, k*c2 are exact in fp32 for small integer k)
def _cw_split():
    import struct
    def chop(x, bits):
        u = struct.unpack('<I', struct.pack('<f', np.float32(x)))[0]
        u &= ~((1 << bits) - 1)
        return struct.unpack('<f', struct.pack('<I', u))[0]
    two_pi = 2 * math.pi
    c1 = chop(two_pi, 12)
    c2 = chop(two_pi - c1, 12)
    c3 = np.float32(two_pi - c1 - c2)
    return float(c1), float(c2), float(c3)

CW1, CW2, CW3 = _cw_split()

_bf16 = ml_dtypes.bfloat16


def _install_axon_hooks():
    """antenv.axon_hooks is absent on this image; shim it so trace=True works."""
    import antenv
    if hasattr(antenv, "axon_hooks"):
        return
    mod = types.ModuleType("antenv.axon_hooks")
    _hook = [None]
    mod.set_axon_ntff_profile_hook = lambda h: _hook.__setitem__(0, h)
    mod.get_axon_ntff_profile_hook = lambda: _hook[0]
    sys.modules["antenv.axon_hooks"] = mod
    antenv.axon_hooks = mod
    try:
        from trn_agent_boot.trn_boot import _ntff_profile_via_ctypes
        mod.set_axon_ntff_profile_hook(
            _ntff_profile_via_ctypes("/opt/axon/libaxon_pjrt.so"))
    except Exception:
        pass


def build_program():
    from concourse import bacc
    import concourse.bass as bass
    import concourse.mybir as mybir
    import concourse.tile as tile
    from concourse.masks import make_identity
    from contextlib import ExitStack

    f32 = mybir.dt.float32
    bf = mybir.dt.bfloat16
    AF = mybir.ActivationFunctionType
    ALU = mybir.AluOpType

    nc = bacc.Bacc("TRN2", target_bir_lowering=False, debug=False,
                   num_devices=N_CORES)

    qx = nc.dram_tensor("qx", [S, DM], bf, kind="ExternalInput").ap()
    kx = nc.dram_tensor("kx", [S, DM], bf, kind="ExternalInput").ap()
    vx = nc.dram_tensor("vx", [S, DM], bf, kind="ExternalInput").ap()
    wqt = nc.dram_tensor("wqt", [DM, DV], bf, kind="ExternalInput").ap()
    wkt = nc.dram_tensor("wkt", [DM, DV], bf, kind="ExternalInput").ap()
    wvt = nc.dram_tensor("wvt", [DM, DV], bf, kind="ExternalInput").ap()
    wot = nc.dram_tensor("wot", [DV, DM], bf, kind="ExternalInput").ap()
    xq = nc.dram_tensor("xq", [S, PHYS], f32, kind="ExternalInput").ap()
    xk = nc.dram_tensor("xk", [S, PHYS], f32, kind="ExternalInput").ap()
    freqs = nc.dram_tensor("freqs", [1, NF], f32, kind="ExternalInput").ap()
    gbq = nc.dram_tensor("gbq", [2, DH], f32, kind="ExternalInput").ap()
    gbk = nc.dram_tensor("gbk", [2, DH], f32, kind="ExternalInput").ap()
    out = nc.dram_tensor("out", [S, DM], f32, kind="ExternalOutput").ap()

    out_t = out.rearrange("(t p) n -> p t n", p=128)       # [128, 16, 1536]
    xq_t = xq.rearrange("(t p) c -> p t c", p=128)         # [128, 16, 3]
    xk_t = xk.rearrange("(t p) c -> p t c", p=128)

    with tile.TileContext(nc) as tc, ExitStack() as ctx:
        consts = ctx.enter_context(tc.tile_pool(name="consts", bufs=1))

        ident = consts.tile([128, 128], bf, tag="ident")
        make_identity(nc, ident)

        freqs_sb = consts.tile([1, NF], f32, tag="freqs1")
        nc.sync.dma_start(out=freqs_sb, in_=freqs)
        freqs_bc = consts.tile([128, NF], f32, tag="freqsbc")
        nc.gpsimd.partition_broadcast(freqs_bc, freqs_sb)

        eps_sb = consts.tile([128, 1], f32, tag="eps")
        nc.vector.memset(eps_sb, LN_EPS)

        # gamma/beta broadcast to all partitions: gb128[p, qk, {gamma,beta}, d]
        gbq_sb = consts.tile([1, 2, DH], f32, tag="gbq")
        nc.sync.dma_start(out=gbq_sb,
                          in_=gbq.rearrange("(o a) d -> o a d", o=1))
        gbk_sb = consts.tile([1, 2, DH], f32, tag="gbk")
        nc.sync.dma_start(out=gbk_sb,
                          in_=gbk.rearrange("(o a) d -> o a d", o=1))
        gb128 = consts.tile([128, 2, 2, DH], f32, tag="gb128")
        nc.gpsimd.partition_broadcast(
            gb128[:, 0].rearrange("p b d -> p (b d)"),
            gbq_sb.rearrange("o b d -> o (b d)"))
        nc.gpsimd.partition_broadcast(
            gb128[:, 1].rearrange("p b d -> p (b d)"),
            gbk_sb.rearrange("o b d -> o (b d)"))

        xq_sb = consts.tile([128, SQ_TILES, PHYS], f32, tag="xq")
        nc.sync.dma_start(out=xq_sb, in_=xq_t)
        xk_sb = consts.tile([128, SQ_TILES, PHYS], f32, tag="xk")
        nc.sync.dma_start(out=xk_sb, in_=xk_t)

        # persistent per-head activations
        heads = ctx.enter_context(tc.tile_pool(name="heads", bufs=1))
        qT = [heads.tile([DH, S], bf, tag=f"qT{h}", name=f"qT{h}")
              for h in range(HG)]
        kT = [heads.tile([DH, S], bf, tag=f"kT{h}", name=f"kT{h}")
              for h in range(HG)]
        # v with a leading ones column per head: [sk_part, sk_tile, head, 1+96]
        v_aug = heads.tile([128, SQ_TILES, HG, 1 + DH], bf, tag="v_aug")
        nc.vector.memset(v_aug[:, :, :, 0:1], 1.0)
        # normalized y^T per head (matmul lhsT needs base partition 0)
        yN = [heads.tile([DH, S], bf, tag=f"yN{h}", name=f"yN{h}")
              for h in range(HG)]

        # ---------------- projections + LN + RoPE + transposes ----------------
        def evict_ln_rope(tensor_idx, t, ps_chunks, work, psT, dst_T):
            """LN + gamma/beta + rope on q/k psum chunks of sq-tile t, then
            per-head PE-transpose into dst_T ([96,2048] bf16 per head)."""
            xln = work.tile([128, 2 * PROJ_CHUNK], f32, tag="xln")
            xln4 = xln.rearrange("p (c h d) -> p (c h) d", c=2, d=DH)
            for c in range(2):
                ps = ps_chunks[c]
                stats = work.tile([128, 4, 6], f32, tag="stats")
                for h4 in range(4):
                    nc.vector.bn_stats(
                        out=stats[:, h4, :],
                        in_=ps.rearrange("p (h d) -> p h d", d=DH)[:, h4, :])
                mv = work.tile([128, 4, 2], f32, tag="mv")
                for h4 in range(4):
                    nc.vector.bn_aggr(out=mv[:, h4, :], in_=stats[:, h4, :])
                rstd = work.tile([128, 4], f32, tag="rstd")
                nc.scalar.activation(out=rstd, in_=mv[:, :, 1],
                                     func=AF.Sqrt, bias=eps_sb, scale=1.0)
                nc.vector.reciprocal(out=rstd, in_=rstd)
                for h4 in range(4):
                    nc.vector.tensor_scalar(
                        out=xln4[:, 4 * c + h4, :],
                        in0=ps.rearrange("p (h d) -> p h d", d=DH)[:, h4, :],
                        scalar1=mv[:, h4, 0:1], scalar2=rstd[:, h4:h4 + 1],
                        op0=ALU.subtract, op1=ALU.mult)
            # gamma/beta (identity for the given data, kept for generality)
            xln3 = xln.rearrange("p (h d) -> p h d", d=DH)
            gammab = gb128[:, tensor_idx, 0, :].rearrange(
                "p (o d) -> p o d", o=1).broadcast_to([128, HG, DH])
            betab = gb128[:, tensor_idx, 1, :].rearrange(
                "p (o d) -> p o d", o=1).broadcast_to([128, HG, DH])
            nc.vector.tensor_tensor(out=xln3, in0=xln3, in1=gammab, op=ALU.mult)
            nc.vector.tensor_tensor(out=xln3, in0=xln3, in1=betab, op=ALU.add)
            # rope angles
            x_sb = xq_sb if tensor_idx == 0 else xk_sb
            theta = work.tile([128, PHYS * NF], f32, tag="theta")
            for p in range(PHYS):
                nc.vector.tensor_scalar_mul(
                    out=theta[:, p * NF:(p + 1) * NF], in0=freqs_bc,
                    scalar1=x_sb[:, t, p:p + 1])
            # range-reduce for ACT Sin (valid domain [-pi, pi]):
            # k = round(theta/2pi) via the fp32 magic-number trick, then
            # Cody-Waite cascade theta - k*2pi, then wrap into [-pi, pi].
            MAGIC = 1.5 * 2.0 ** 23
            kmul = work.tile([128, PHYS * NF], f32, tag="kmul")
            nc.vector.tensor_scalar(out=kmul, in0=theta,
                                    scalar1=1.0 / (2 * math.pi),
                                    scalar2=MAGIC, op0=ALU.mult, op1=ALU.add)
            nc.vector.tensor_single_scalar(out=kmul, in_=kmul, scalar=MAGIC,
                                           op=ALU.subtract)
            nc.vector.cody_waite_cascade(out=theta, x=theta, k=kmul,
                                         c1=CW1, c2=CW2, c3=CW3)
            ts_ = kmul   # kmul's value is dead; reuse its slot
            tcs = work.tile([128, PHYS * NF], f32, tag="tcs")
            nc.vector.add_range_wrap(out=ts_, in_=theta, shift=0.0,
                                     bound=math.pi, period=2 * math.pi)
            nc.vector.add_range_wrap(out=tcs, in_=theta, shift=math.pi / 2,
                                     bound=math.pi, period=2 * math.pi)
            cos48 = work.tile([128, PHYS * NF], f32, tag="cos48")
            sin48 = work.tile([128, PHYS * NF], f32, tag="sin48")
            nc.scalar.activation(out=cos48, in_=tcs, func=AF.Sin,
                                 bias=0.0, scale=1.0)
            nc.scalar.activation(out=sin48, in_=ts_, func=AF.Sin,
                                 bias=0.0, scale=1.0)
            # rope: pairs are (even, odd) along each head's 96 dims
            xe = xln.rearrange("p (h d) -> p h d", d=DH)[:, :, 0::2]  # [128,8,48]
            xo = xln.rearrange("p (h d) -> p h d", d=DH)[:, :, 1::2]
            cosb = cos48.rearrange("p (o f) -> p o f", o=1).broadcast_to(
                [128, HG, PHYS * NF])
            sinb = sin48.rearrange("p (o f) -> p o f", o=1).broadcast_to(
                [128, HG, PHYS * NF])
            t1 = work.tile([128, HG, PHYS * NF], f32, tag="t1")
            t2 = work.tile([128, HG, PHYS * NF], f32, tag="t2")
            rot = work.tile([128, DV], bf, tag="rot")
            rote = rot.rearrange("p (h d) -> p h d", d=DH)[:, :, 0::2]
            roto = rot.rearrange("p (h d) -> p h d", d=DH)[:, :, 1::2]
            nc.vector.tensor_mul(out=t1, in0=xe, in1=cosb)
            nc.vector.tensor_mul(out=t2, in0=xo, in1=sinb)
            nc.vector.tensor_sub(out=rote, in0=t1, in1=t2)
            nc.vector.tensor_mul(out=t1, in0=xe, in1=sinb)
            nc.vector.tensor_mul(out=t2, in0=xo, in1=cosb)
            nc.vector.tensor_add(out=roto, in0=t1, in1=t2)
            # transpose each head's [128, 96] block into dst_T[h][:, t*128:...]
            for h in range(HG):
                tp = psT.tile([DH, 128], bf, tag="tp")
                nc.tensor.transpose(out=tp, in_=rot[:, h * DH:(h + 1) * DH],
                                    identity=ident)
                nc.scalar.copy(out=dst_T[h][:, t * 128:(t + 1) * 128], in_=tp)

        with ExitStack() as proj_ctx:
            xT_pool = proj_ctx.enter_context(tc.tile_pool(name="xT", bufs=1))
            w_pool = proj_ctx.enter_context(tc.tile_pool(name="w", bufs=1))
            work = proj_ctx.enter_context(tc.tile_pool(name="work", bufs=2))
            ps_pool = proj_ctx.enter_context(
                tc.tile_pool(name="ps_proj", bufs=4, space="PSUM"))
            psT_pool = proj_ctx.enter_context(
                tc.tile_pool(name="ps_tp", bufs=4, space="PSUM"))

            for tensor_idx, (x_dram, w_dram) in enumerate(
                    [(qx, wqt), (kx, wkt), (vx, wvt)]):
                xT = xT_pool.tile([128, K_TILES, S], bf, tag="xT")
                for j in range(K_TILES):
                    nc.sync.dma_start_transpose(
                        out=xT[:, j, :], in_=x_dram[:, j * 128:(j + 1) * 128])
                w_sb = w_pool.tile([128, K_TILES, DV], bf, tag="w")
                nc.sync.dma_start(
                    out=w_sb, in_=w_dram.rearrange("(j p) n -> p j n", p=128))

                for t in range(SQ_TILES):
                    ps_chunks = []
                    for c in range(2):
                        ps = ps_pool.tile([128, PROJ_CHUNK], f32, tag="proj")
                        for j in range(K_TILES):
                            nc.tensor.matmul(
                                ps, lhsT=xT[:, j, t * 128:(t + 1) * 128],
                                rhs=w_sb[:, j,
                                         c * PROJ_CHUNK:(c + 1) * PROJ_CHUNK],
                                start=(j == 0), stop=(j == K_TILES - 1))
                        ps_chunks.append(ps)
                    if tensor_idx < 2:
                        evict_ln_rope(tensor_idx, t, ps_chunks, work, psT_pool,
                                      qT if tensor_idx == 0 else kT)
                    else:
                        for c in range(2):
                            nc.vector.tensor_copy(
                                out=v_aug[:, t, 4 * c:4 * c + 4, 1:],
                                in_=ps_chunks[c].rearrange(
                                    "p (h d) -> p h d", d=DH))

        # ---------------- attention ----------------
        with ExitStack() as att_ctx:
            e_pool = att_ctx.enter_context(tc.tile_pool(name="E", bufs=3))
            s_pool = att_ctx.enter_context(
                tc.tile_pool(name="ps_s", bufs=3, space="PSUM"))
            y_pool = att_ctx.enter_context(
                tc.tile_pool(name="ps_y", bufs=1, space="PSUM"))
            nrm = att_ctx.enter_context(tc.tile_pool(name="nrm", bufs=2))

            for h in range(HG):
                y_ps = [y_pool.tile([1 + DH, 512], f32, tag=f"y{c}", name=f"y_ps{c}")
                        for c in range(4)]
                for sk in range(SQ_TILES):
                    e_tile = e_pool.tile([128, S], bf, tag="E")
                    for c4 in range(4):
                        s_ps = s_pool.tile([128, 512], f32, tag="S")
                        nc.tensor.matmul(
                            s_ps, lhsT=kT[h][:, sk * 128:(sk + 1) * 128],
                            rhs=qT[h][:, c4 * 512:(c4 + 1) * 512],
                            start=True, stop=True)
                        nc.scalar.activation(
                            out=e_tile[:, c4 * 512:(c4 + 1) * 512], in_=s_ps,
                            func=AF.Exp, scale=SCALE)
                        nc.tensor.matmul(
                            y_ps[c4], lhsT=v_aug[:, sk, h, :],
                            rhs=e_tile[:, c4 * 512:(c4 + 1) * 512],
                            start=(sk == 0), stop=(sk == SQ_TILES - 1))
                yst = nrm.tile([1 + DH, S], bf, tag="yst")
                for c4 in range(4):
                    r1 = nrm.tile([1, 512], f32, tag="r1")
                    nc.vector.reciprocal(out=r1, in_=y_ps[c4][0:1, :])
                    rbc = nrm.tile([1 + DH, 512], f32, tag="rbc")
                    nc.gpsimd.partition_broadcast(rbc, r1)
                    nc.vector.tensor_tensor(
                        out=yst[:, c4 * 512:(c4 + 1) * 512],
                        in0=y_ps[c4], in1=rbc, op=ALU.mult)
                # shift rows 1..96 down to partitions 0..95 (DMA partition remap)
                nc.sync.dma_start(out=yN[h], in_=yst[1:, :])

        # ---------------- output projection ----------------
        with ExitStack() as op_ctx:
            wo_pool = op_ctx.enter_context(tc.tile_pool(name="wo", bufs=1))
            o_pool = op_ctx.enter_context(
                tc.tile_pool(name="ps_o", bufs=3, space="PSUM"))
            oev = op_ctx.enter_context(tc.tile_pool(name="oev", bufs=4))

            woT = [wo_pool.tile([DH, DM], bf, tag=f"wo{h}", name=f"woT{h}")
                   for h in range(HG)]
            for h in range(HG):
                nc.sync.dma_start(out=woT[h],
                                  in_=wot[h * DH:(h + 1) * DH, :])
            for t in range(SQ_TILES):
                for c3 in range(3):
                    o_ps = o_pool.tile([128, 512], f32, tag="o")
                    for h in range(HG):
                        nc.tensor.matmul(
                            o_ps, lhsT=yN[h][:, t * 128:(t + 1) * 128],
                            rhs=woT[h][:, c3 * 512:(c3 + 1) * 512],
                            start=(h == 0), stop=(h == HG - 1))
                    o_sb = oev.tile([128, 512], f32, tag="osb")
                    nc.scalar.copy(out=o_sb, in_=o_ps)
                    nc.sync.dma_start(out=out_t[:, t, c3 * 512:(c3 + 1) * 512],
                                      in_=o_sb)

    nc.compile()
    return nc


_PROGRAM = None


def _get_program():
    global _PROGRAM
    if _PROGRAM is None:
        _PROGRAM = build_program()
    return _PROGRAM


def make_in_maps(qx, kx, vx, x_q, x_k, Wq, Wk, Wv, Wo, q_gamma, q_beta,
                 k_gamma, k_beta):
    freqs = np.exp(np.linspace(MIN_LF, MAX_LF, NF)).astype(np.float32)
    in_maps = []
    for core in range(N_CORES):
        b, g = core // 2, core % 2
        rows = slice(g * DV, (g + 1) * DV)
        in_maps.append({
            "qx": np.ascontiguousarray(qx[b]).astype(_bf16),
            "kx": np.ascontiguousarray(kx[b]).astype(_bf16),
            "vx": np.ascontiguousarray(vx[b]).astype(_bf16),
            "wqt": np.ascontiguousarray(Wq[rows].T).astype(_bf16),
            "wkt": np.ascontiguousarray(Wk[rows].T).astype(_bf16),
            "wvt": np.ascontiguousarray(Wv[rows].T).astype(_bf16),
            "wot": np.ascontiguousarray(Wo[:, rows].T).astype(_bf16),
            "xq": np.ascontiguousarray(x_q[b]).astype(np.float32),
            "xk": np.ascontiguousarray(x_k[b]).astype(np.float32),
            "freqs": freqs[None, :],
            "gbq": np.stack([q_gamma, q_beta]).astype(np.float32),
            "gbk": np.stack([k_gamma, k_beta]).astype(np.float32),
        })
    return in_maps


LAST_EXEC_TIME_NS = None


def kernel(qx, kx, vx, x_q, x_k, Wq, Wk, Wv, Wo, q_gamma, q_beta,
           k_gamma, k_beta):
    global LAST_EXEC_TIME_NS
    import os
    _install_axon_hooks()
    from concourse.bass_utils import run_bass_kernel_spmd

    nc = _get_program()
    in_maps = make_in_maps(np.asarray(qx), np.asarray(kx), np.asarray(vx),
                           np.asarray(x_q), np.asarray(x_k), np.asarray(Wq),
                           np.asarray(Wk), np.asarray(Wv), np.asarray(Wo),
                           np.asarray(q_gamma), np.asarray(q_beta),
                           np.asarray(k_gamma), np.asarray(k_beta))
    trace = bool(int(os.environ.get("KERNEL_TRACE", "0")))
    res = run_bass_kernel_spmd(nc, in_maps, list(range(N_CORES)), trace=trace)
    LAST_EXEC_TIME_NS = res.exec_time_ns
    outv = np.empty((B, S, DM), np.float32)
    for b in range(B):
        outv[b] = res.results[2 * b]["out"] + res.results[2 * b + 1]["out"]
    return outv


# revision 24
# speedup vs baseline: 1.1021x; 1.1021x over previous
"""Self-contained Trainium2 Bass kernel for MultiHeadAttention with QK-layernorm
and physical-coordinate RoPE.

Sharding: 8 cores = 4 batches x 2 head-groups (8 heads each).  Each core
computes its batch's projections for its head group, attention, and a partial
output projection (row-sharded Wo); the host sums the two partials per batch.
"""

import math
import sys
import types

import numpy as np
import ml_dtypes

# ---- problem constants (hardcoded; kernel.py must not read spec/reference) ----
B, S, DM = 4, 2048, 1536
H_TOT, DH = 16, 96
HG = 8                      # heads per core
DV = HG * DH                # 768 per-core projection width
PHYS, NF = 3, 16            # phys dims, freqs
MIN_LF, MAX_LF = -5.0, 3.0
LN_EPS = 1e-5
N_CORES = 8

SQ_TILES = S // 128         # 16
K_TILES = DM // 128         # 12
PROJ_CHUNK = 384            # 4 heads worth of dv per psum chunk
SCALE = 1.0 / math.sqrt(DH)

# Cody-Waite 3-term split of 2*pi (c1/c2 have trailing mantissa zeroed so
# k*c1, k*c2 are exact in fp32 for small integer k)
def _cw_split():
    import struct
    def chop(x, bits):
        u = struct.unpack('<I', struct.pack('<f', np.float32(x)))[0]
        u &= ~((1 << bits) - 1)
        return struct.unpack('<f', struct.pack('<I', u))[0]
    two_pi = 2 * math.pi
    c1 = chop(two_pi, 12)
    c2 = chop(two_pi - c1, 12)
    c3 = np.float32(two_pi - c1 - c2)
    return float(c1), float(c2), float(c3)

CW1, CW2, CW3 = _cw_split()

_bf16 = ml_dtypes.bfloat16


def _install_axon_hooks():
    """antenv.axon_hooks is absent on this image; shim it so trace=True works."""
    import antenv
    if hasattr(antenv, "axon_hooks"):
        return
    mod = types.ModuleType("antenv.axon_hooks")
    _hook = [None]
    mod.set_axon_ntff_profile_hook = lambda h: _hook.__setitem__(0, h)
    mod.get_axon_ntff_profile_hook = lambda: _hook[0]
    sys.modules["antenv.axon_hooks"] = mod
    antenv.axon_hooks = mod
    try:
        from trn_agent_boot.trn_boot import _ntff_profile_via_ctypes
        mod.set_axon_ntff_profile_hook(
            _ntff_profile_via_ctypes("/opt/axon/libaxon_pjrt.so"))
    except Exception:
        pass


def build_program():
    from concourse import bacc
    import concourse.bass as bass
    import concourse.mybir as mybir
    import concourse.tile as tile
    from concourse.masks import make_identity
    from contextlib import ExitStack

    f32 = mybir.dt.float32
    bf = mybir.dt.bfloat16
    AF = mybir.ActivationFunctionType
    ALU = mybir.AluOpType

    nc = bacc.Bacc("TRN2", target_bir_lowering=False, debug=False,
                   num_devices=N_CORES)

    qx = nc.dram_tensor("qx", [S, DM], bf, kind="ExternalInput").ap()
    kx = nc.dram_tensor("kx", [S, DM], bf, kind="ExternalInput").ap()
    vx = nc.dram_tensor("vx", [S, DM], bf, kind="ExternalInput").ap()
    wqt = nc.dram_tensor("wqt", [DM, DV], bf, kind="ExternalInput").ap()
    wkt = nc.dram_tensor("wkt", [DM, DV], bf, kind="ExternalInput").ap()
    wvt = nc.dram_tensor("wvt", [DM, DV], bf, kind="ExternalInput").ap()
    wot = nc.dram_tensor("wot", [DV, DM], bf, kind="ExternalInput").ap()
    xq = nc.dram_tensor("xq", [S, PHYS], f32, kind="ExternalInput").ap()
    xk = nc.dram_tensor("xk", [S, PHYS], f32, kind="ExternalInput").ap()
    freqs = nc.dram_tensor("freqs", [1, NF], f32, kind="ExternalInput").ap()
    gbq = nc.dram_tensor("gbq", [2, DH], f32, kind="ExternalInput").ap()
    gbk = nc.dram_tensor("gbk", [2, DH], f32, kind="ExternalInput").ap()
    out = nc.dram_tensor("out", [S, DM], f32, kind="ExternalOutput").ap()

    out_t = out.rearrange("(t p) n -> p t n", p=128)       # [128, 16, 1536]
    xq_t = xq.rearrange("(t p) c -> p t c", p=128)         # [128, 16, 3]
    xk_t = xk.rearrange("(t p) c -> p t c", p=128)

    with tile.TileContext(nc) as tc, ExitStack() as ctx:
        consts = ctx.enter_context(tc.tile_pool(name="consts", bufs=1))

        ident = consts.tile([128, 128], bf, tag="ident")
        make_identity(nc, ident)

        freqs_sb = consts.tile([1, NF], f32, tag="freqs1")
        nc.sync.dma_start(out=freqs_sb, in_=freqs)
        freqs_bc = consts.tile([128, NF], f32, tag="freqsbc")
        nc.gpsimd.partition_broadcast(freqs_bc, freqs_sb)

        eps_sb = consts.tile([128, 1], f32, tag="eps")
        nc.vector.memset(eps_sb, LN_EPS)

        # gamma/beta broadcast to all partitions: gb128[p, qk, {gamma,beta}, d]
        gbq_sb = consts.tile([1, 2, DH], f32, tag="gbq")
        nc.sync.dma_start(out=gbq_sb,
                          in_=gbq.rearrange("(o a) d -> o a d", o=1))
        gbk_sb = consts.tile([1, 2, DH], f32, tag="gbk")
        nc.sync.dma_start(out=gbk_sb,
                          in_=gbk.rearrange("(o a) d -> o a d", o=1))
        gb128 = consts.tile([128, 2, 2, DH], f32, tag="gb128")
        nc.gpsimd.partition_broadcast(
            gb128[:, 0].rearrange("p b d -> p (b d)"),
            gbq_sb.rearrange("o b d -> o (b d)"))
        nc.gpsimd.partition_broadcast(
            gb128[:, 1].rearrange("p b d -> p (b d)"),
            gbk_sb.rearrange("o b d -> o (b d)"))

        xq_sb = consts.tile([128, SQ_TILES, PHYS], f32, tag="xq")
        nc.sync.dma_start(out=xq_sb, in_=xq_t)
        xk_sb = consts.tile([128, SQ_TILES, PHYS], f32, tag="xk")
        nc.sync.dma_start(out=xk_sb, in_=xk_t)

        # persistent per-head activations
        heads = ctx.enter_context(tc.tile_pool(name="heads", bufs=1))
        qT_all = heads.tile([DH, HG, S], bf, tag="qT_all")
        kT_all = heads.tile([DH, HG, S], bf, tag="kT_all")
        # v with a leading ones column per head: [sk_part, sk_tile, head, 1+96]
        v_aug = heads.tile([128, SQ_TILES, HG, 1 + DH], bf, tag="v_aug")
        nc.vector.memset(v_aug[:, :, :, 0:1], 1.0)
        # normalized y^T per head (matmul lhsT needs base partition 0)
        yN_all = heads.tile([DH, HG, S], bf, tag="yN_all")

        # ---------------- projections + LN + RoPE + transposes ----------------
        def evict_ln_rope(tensor_idx, t, ps_chunks, work, psT, dst_T):
            """LN + gamma/beta + rope on q/k psum chunks of sq-tile t, then
            per-head PE-transpose into dst_T ([96,2048] bf16 per head)."""
            xln = work.tile([128, 2 * PROJ_CHUNK], f32, tag="xln")
            xln4 = xln.rearrange("p (c h d) -> p (c h) d", c=2, d=DH)
            for c in range(2):
                ps = ps_chunks[c]
                stats = work.tile([128, 4, 6], f32, tag="stats")
                for h4 in range(4):
                    nc.vector.bn_stats(
                        out=stats[:, h4, :],
                        in_=ps.rearrange("p (h d) -> p h d", d=DH)[:, h4, :])
                mv = work.tile([128, 4, 2], f32, tag="mv")
                for h4 in range(4):
                    nc.vector.bn_aggr(out=mv[:, h4, :], in_=stats[:, h4, :])
                rstd = work.tile([128, 4], f32, tag="rstd")
                nc.scalar.activation(out=rstd, in_=mv[:, :, 1],
                                     func=AF.Sqrt, bias=eps_sb, scale=1.0)
                nc.vector.reciprocal(out=rstd, in_=rstd)
                for h4 in range(4):
                    nc.vector.tensor_scalar(
                        out=xln4[:, 4 * c + h4, :],
                        in0=ps.rearrange("p (h d) -> p h d", d=DH)[:, h4, :],
                        scalar1=mv[:, h4, 0:1], scalar2=rstd[:, h4:h4 + 1],
                        op0=ALU.subtract, op1=ALU.mult)
            # gamma/beta (identity for the given data, kept for generality)
            xln3 = xln.rearrange("p (h d) -> p h d", d=DH)
            gammab = gb128[:, tensor_idx, 0, :].rearrange(
                "p (o d) -> p o d", o=1).broadcast_to([128, HG, DH])
            betab = gb128[:, tensor_idx, 1, :].rearrange(
                "p (o d) -> p o d", o=1).broadcast_to([128, HG, DH])
            nc.vector.tensor_tensor(out=xln3, in0=xln3, in1=gammab, op=ALU.mult)
            nc.vector.tensor_tensor(out=xln3, in0=xln3, in1=betab, op=ALU.add)
            # rope angles
            x_sb = xq_sb if tensor_idx == 0 else xk_sb
            theta = work.tile([128, PHYS * NF], f32, tag="theta")
            for p in range(PHYS):
                nc.vector.tensor_scalar_mul(
                    out=theta[:, p * NF:(p + 1) * NF], in0=freqs_bc,
                    scalar1=x_sb[:, t, p:p + 1])
            # range-reduce for ACT Sin (valid domain [-pi, pi]):
            # k = round(theta/2pi) via the fp32 magic-number trick, then
            # Cody-Waite cascade theta - k*2pi, then wrap into [-pi, pi].
            MAGIC = 1.5 * 2.0 ** 23
            kmul = work.tile([128, PHYS * NF], f32, tag="kmul")
            nc.vector.tensor_scalar(out=kmul, in0=theta,
                                    scalar1=1.0 / (2 * math.pi),
                                    scalar2=MAGIC, op0=ALU.mult, op1=ALU.add)
            nc.vector.tensor_single_scalar(out=kmul, in_=kmul, scalar=MAGIC,
                                           op=ALU.subtract)
            nc.vector.cody_waite_cascade(out=theta, x=theta, k=kmul,
                                         c1=CW1, c2=CW2, c3=CW3)
            ts_ = kmul   # kmul's value is dead; reuse its slot
            tcs = work.tile([128, PHYS * NF], f32, tag="tcs")
            nc.vector.add_range_wrap(out=ts_, in_=theta, shift=0.0,
                                     bound=math.pi, period=2 * math.pi)
            nc.vector.add_range_wrap(out=tcs, in_=theta, shift=math.pi / 2,
                                     bound=math.pi, period=2 * math.pi)
            cos48 = work.tile([128, PHYS * NF], f32, tag="cos48")
            sin48 = work.tile([128, PHYS * NF], f32, tag="sin48")
            nc.scalar.activation(out=cos48, in_=tcs, func=AF.Sin,
                                 bias=0.0, scale=1.0)
            nc.scalar.activation(out=sin48, in_=ts_, func=AF.Sin,
                                 bias=0.0, scale=1.0)
            # rope: pairs are (even, odd) along each head's 96 dims
            xe = xln.rearrange("p (h d) -> p h d", d=DH)[:, :, 0::2]  # [128,8,48]
            xo = xln.rearrange("p (h d) -> p h d", d=DH)[:, :, 1::2]
            cosb = cos48.rearrange("p (o f) -> p o f", o=1).broadcast_to(
                [128, HG, PHYS * NF])
            sinb = sin48.rearrange("p (o f) -> p o f", o=1).broadcast_to(
                [128, HG, PHYS * NF])
            t1 = work.tile([128, HG, PHYS * NF], f32, tag="t1")
            t2 = work.tile([128, HG, PHYS * NF], f32, tag="t2")
            rot = work.tile([128, DV], bf, tag="rot")
            rote = rot.rearrange("p (h d) -> p h d", d=DH)[:, :, 0::2]
            roto = rot.rearrange("p (h d) -> p h d", d=DH)[:, :, 1::2]
            nc.vector.tensor_mul(out=t1, in0=xe, in1=cosb)
            nc.vector.tensor_mul(out=t2, in0=xo, in1=sinb)
            nc.vector.tensor_sub(out=rote, in0=t1, in1=t2)
            nc.vector.tensor_mul(out=t1, in0=xe, in1=sinb)
            nc.vector.tensor_mul(out=t2, in0=xo, in1=cosb)
            nc.vector.tensor_add(out=roto, in0=t1, in1=t2)
            # transpose each head's [128, 96] block; batch 4 heads per psum
            # tile so the psum->SBUF eviction is one op per 4 heads
            for c in range(2):
                tp = psT.tile([DH, 4, 128], bf, tag="tp")
                for i in range(4):
                    nc.tensor.transpose(
                        out=tp[:, i, :],
                        in_=rot[:, (4 * c + i) * DH:(4 * c + i + 1) * DH],
                        identity=ident)
                nc.scalar.copy(
                    out=dst_T[:, 4 * c:4 * c + 4, t * 128:(t + 1) * 128],
                    in_=tp)

        with ExitStack() as proj_ctx:
            xT_pool = proj_ctx.enter_context(tc.tile_pool(name="xT", bufs=2))
            w_pool = proj_ctx.enter_context(tc.tile_pool(name="w", bufs=1))
            work = proj_ctx.enter_context(tc.tile_pool(name="work", bufs=2))
            ps_pool = proj_ctx.enter_context(
                tc.tile_pool(name="ps_proj", bufs=4, space="PSUM"))
            psT_pool = proj_ctx.enter_context(
                tc.tile_pool(name="ps_tp", bufs=4, space="PSUM"))

            SH = S // 2
            for tensor_idx, (x_dram, w_dram) in enumerate(
                    [(qx, wqt), (kx, wkt), (vx, wvt)]):
                w_sb = w_pool.tile([128, K_TILES, DV], bf, tag="w")
                nc.sync.dma_start(
                    out=w_sb, in_=w_dram.rearrange("(j p) n -> p j n", p=128))
                for half in range(2):
                    xT = xT_pool.tile([128, K_TILES, SH], bf, tag="xT")
                    for j in range(K_TILES):
                        nc.sync.dma_start_transpose(
                            out=xT[:, j, :],
                            in_=x_dram[half * SH:(half + 1) * SH,
                                       j * 128:(j + 1) * 128])
                    for tl in range(SH // 128):
                        t = half * (SH // 128) + tl
                        ps_chunks = []
                        for c in range(2):
                            ps = ps_pool.tile([128, PROJ_CHUNK], f32, tag="proj")
                            for j in range(K_TILES):
                                nc.tensor.matmul(
                                    ps, lhsT=xT[:, j, tl * 128:(tl + 1) * 128],
                                    rhs=w_sb[:, j,
                                             c * PROJ_CHUNK:(c + 1) * PROJ_CHUNK],
                                    start=(j == 0), stop=(j == K_TILES - 1))
                            ps_chunks.append(ps)
                        if tensor_idx < 2:
                            evict_ln_rope(tensor_idx, t, ps_chunks, work,
                                          psT_pool,
                                          qT_all if tensor_idx == 0 else kT_all)
                        else:
                            for c in range(2):
                                nc.vector.tensor_copy(
                                    out=v_aug[:, t, 4 * c:4 * c + 4, 1:],
                                    in_=ps_chunks[c].rearrange(
                                        "p (h d) -> p h d", d=DH))

        # ---------------- attention ----------------
        with ExitStack() as att_ctx:
            e_pool = att_ctx.enter_context(tc.tile_pool(name="E", bufs=3))
            s_pool = att_ctx.enter_context(
                tc.tile_pool(name="ps_s", bufs=2, space="PSUM"))
            y_pool = att_ctx.enter_context(
                tc.tile_pool(name="ps_y", bufs=1, space="PSUM"))
            nrm = att_ctx.enter_context(tc.tile_pool(name="nrm", bufs=2))

            for h in range(HG):
                y_ps = [y_pool.tile([1 + DH, 512], f32, tag=f"y{c}", name=f"y_ps{c}")
                        for c in range(4)]
                for sk in range(SQ_TILES):
                    e_tile = e_pool.tile([128, S], bf, tag="E")
                    kslice = kT_all[:, h, sk * 128:(sk + 1) * 128]
                    for cc in range(2):
                        s_ps = s_pool.tile([128, 2, 512], f32, tag="S")
                        for i in range(2):
                            c4 = 2 * cc + i
                            nc.tensor.matmul(
                                s_ps[:, i, :], lhsT=kslice,
                                rhs=qT_all[:, h, c4 * 512:(c4 + 1) * 512],
                                start=True, stop=True)
                        nc.scalar.activation(
                            out=e_tile[:, cc * 1024:(cc + 1) * 1024],
                            in_=s_ps.rearrange("p a b -> p (a b)"),
                            func=AF.Exp, scale=SCALE)
                        for i in range(2):
                            c4 = 2 * cc + i
                            nc.tensor.matmul(
                                y_ps[c4], lhsT=v_aug[:, sk, h, :],
                                rhs=e_tile[:, c4 * 512:(c4 + 1) * 512],
                                start=(sk == 0), stop=(sk == SQ_TILES - 1))
                yst = nrm.tile([1 + DH, S], bf, tag="yst")
                for c4 in range(4):
                    r1 = nrm.tile([1, 512], f32, tag="r1")
                    nc.vector.reciprocal(out=r1, in_=y_ps[c4][0:1, :])
                    rbc = nrm.tile([1 + DH, 512], f32, tag="rbc")
                    nc.gpsimd.partition_broadcast(rbc, r1)
                    nc.vector.tensor_tensor(
                        out=yst[:, c4 * 512:(c4 + 1) * 512],
                        in0=y_ps[c4], in1=rbc, op=ALU.mult)
                # shift rows 1..96 down to partitions 0..95 (DMA partition remap)
                nc.sync.dma_start(out=yN_all[:, h, :], in_=yst[1:, :])

        # ---------------- output projection ----------------
        with ExitStack() as op_ctx:
            wo_pool = op_ctx.enter_context(tc.tile_pool(name="wo", bufs=1))
            o_pool = op_ctx.enter_context(
                tc.tile_pool(name="ps_o", bufs=3, space="PSUM"))
            oev = op_ctx.enter_context(tc.tile_pool(name="oev", bufs=4))

            woT = [wo_pool.tile([DH, DM], bf, tag=f"wo{h}", name=f"woT{h}")
                   for h in range(HG)]
            for h in range(HG):
                nc.sync.dma_start(out=woT[h],
                                  in_=wot[h * DH:(h + 1) * DH, :])
            for t in range(SQ_TILES):
                for c3 in range(3):
                    o_ps = o_pool.tile([128, 512], f32, tag="o")
                    for h in range(HG):
                        nc.tensor.matmul(
                            o_ps, lhsT=yN_all[:, h, t * 128:(t + 1) * 128],
                            rhs=woT[h][:, c3 * 512:(c3 + 1) * 512],
                            start=(h == 0), stop=(h == HG - 1))
                    o_sb = oev.tile([128, 512], f32, tag="osb")
                    nc.scalar.copy(out=o_sb, in_=o_ps)
                    nc.sync.dma_start(out=out_t[:, t, c3 * 512:(c3 + 1) * 512],
                                      in_=o_sb)

    nc.compile()
    return nc


_PROGRAM = None


def _get_program():
    global _PROGRAM
    if _PROGRAM is None:
        _PROGRAM = build_program()
    return _PROGRAM


def make_in_maps(qx, kx, vx, x_q, x_k, Wq, Wk, Wv, Wo, q_gamma, q_beta,
                 k_gamma, k_beta):
    freqs = np.exp(np.linspace(MIN_LF, MAX_LF, NF)).astype(np.float32)
    in_maps = []
    for core in range(N_CORES):
        b, g = core // 2, core % 2
        rows = slice(g * DV, (g + 1) * DV)
        in_maps.append({
            "qx": np.ascontiguousarray(qx[b]).astype(_bf16),
            "kx": np.ascontiguousarray(kx[b]).astype(_bf16),
            "vx": np.ascontiguousarray(vx[b]).astype(_bf16),
            "wqt": np.ascontiguousarray(Wq[rows].T).astype(_bf16),
            "wkt": np.ascontiguousarray(Wk[rows].T).astype(_bf16),
            "wvt": np.ascontiguousarray(Wv[rows].T).astype(_bf16),
            "wot": np.ascontiguousarray(Wo[:, rows].T).astype(_bf16),
            "xq": np.ascontiguousarray(x_q[b]).astype(np.float32),
            "xk": np.ascontiguousarray(x_k[b]).astype(np.float32),
            "freqs": freqs[None, :],
            "gbq": np.stack([q_gamma, q_beta]).astype(np.float32),
            "gbk": np.stack([k_gamma, k_beta]).astype(np.float32),
        })
    return in_maps


LAST_EXEC_TIME_NS = None


def kernel(qx, kx, vx, x_q, x_k, Wq, Wk, Wv, Wo, q_gamma, q_beta,
           k_gamma, k_beta):
    global LAST_EXEC_TIME_NS
    import os
    _install_axon_hooks()
    from concourse.bass_utils import run_bass_kernel_spmd

    nc = _get_program()
    in_maps = make_in_maps(np.asarray(qx), np.asarray(kx), np.asarray(vx),
                           np.asarray(x_q), np.asarray(x_k), np.asarray(Wq),
                           np.asarray(Wk), np.asarray(Wv), np.asarray(Wo),
                           np.asarray(q_gamma), np.asarray(q_beta),
                           np.asarray(k_gamma), np.asarray(k_beta))
    trace = bool(int(os.environ.get("KERNEL_TRACE", "0")))
    res = run_bass_kernel_spmd(nc, in_maps, list(range(N_CORES)), trace=trace)
    LAST_EXEC_TIME_NS = res.exec_time_ns
    outv = np.empty((B, S, DM), np.float32)
    for b in range(B):
        outv[b] = res.results[2 * b]["out"] + res.results[2 * b + 1]["out"]
    return outv


# revision 25
# speedup vs baseline: 1.1177x; 1.0141x over previous
"""Self-contained Trainium2 Bass kernel for MultiHeadAttention with QK-layernorm
and physical-coordinate RoPE.

Sharding: 8 cores = 4 batches x 2 head-groups (8 heads each).  Each core
computes its batch's projections for its head group, attention, and a partial
output projection (row-sharded Wo); the host sums the two partials per batch.
"""

import math
import sys
import types

import numpy as np
import ml_dtypes

# ---- problem constants (hardcoded; kernel.py must not read spec/reference) ----
B, S, DM = 4, 2048, 1536
H_TOT, DH = 16, 96
HG = 8                      # heads per core
DV = HG * DH                # 768 per-core projection width
PHYS, NF = 3, 16            # phys dims, freqs
MIN_LF, MAX_LF = -5.0, 3.0
LN_EPS = 1e-5
N_CORES = 8

SQ_TILES = S // 128         # 16
K_TILES = DM // 128         # 12
PROJ_CHUNK = 384            # 4 heads worth of dv per psum chunk
SCALE = 1.0 / math.sqrt(DH)

# Cody-Waite 3-term split of 2*pi (c1/c2 have trailing mantissa zeroed so
# k*c1, k*c2 are exact in fp32 for small integer k)
def _cw_split():
    import struct
    def chop(x, bits):
        u = struct.unpack('<I', struct.pack('<f', np.float32(x)))[0]
        u &= ~((1 << bits) - 1)
        return struct.unpack('<f', struct.pack('<I', u))[0]
    two_pi = 2 * math.pi
    c1 = chop(two_pi, 12)
    c2 = chop(two_pi - c1, 12)
    c3 = np.float32(two_pi - c1 - c2)
    return float(c1), float(c2), float(c3)

CW1, CW2, CW3 = _cw_split()

_bf16 = ml_dtypes.bfloat16


def _install_axon_hooks():
    """antenv.axon_hooks is absent on this image; shim it so trace=True works."""
    import antenv
    if hasattr(antenv, "axon_hooks"):
        return
    mod = types.ModuleType("antenv.axon_hooks")
    _hook = [None]
    mod.set_axon_ntff_profile_hook = lambda h: _hook.__setitem__(0, h)
    mod.get_axon_ntff_profile_hook = lambda: _hook[0]
    sys.modules["antenv.axon_hooks"] = mod
    antenv.axon_hooks = mod
    try:
        from trn_agent_boot.trn_boot import _ntff_profile_via_ctypes
        mod.set_axon_ntff_profile_hook(
            _ntff_profile_via_ctypes("/opt/axon/libaxon_pjrt.so"))
    except Exception:
        pass


def build_program():
    from concourse import bacc
    import concourse.bass as bass
    import concourse.mybir as mybir
    import concourse.tile as tile
    from concourse.masks import make_identity
    from contextlib import ExitStack

    f32 = mybir.dt.float32
    bf = mybir.dt.bfloat16
    AF = mybir.ActivationFunctionType
    ALU = mybir.AluOpType

    nc = bacc.Bacc("TRN2", target_bir_lowering=False, debug=False,
                   num_devices=N_CORES)

    qx = nc.dram_tensor("qx", [S, DM], bf, kind="ExternalInput").ap()
    kx = nc.dram_tensor("kx", [S, DM], bf, kind="ExternalInput").ap()
    vx = nc.dram_tensor("vx", [S, DM], bf, kind="ExternalInput").ap()
    wqt = nc.dram_tensor("wqt", [DM, DV], bf, kind="ExternalInput").ap()
    wkt = nc.dram_tensor("wkt", [DM, DV], bf, kind="ExternalInput").ap()
    wvt = nc.dram_tensor("wvt", [DM, DV], bf, kind="ExternalInput").ap()
    wot = nc.dram_tensor("wot", [DV, DM], bf, kind="ExternalInput").ap()
    xq = nc.dram_tensor("xq", [S, PHYS], f32, kind="ExternalInput").ap()
    xk = nc.dram_tensor("xk", [S, PHYS], f32, kind="ExternalInput").ap()
    freqs = nc.dram_tensor("freqs", [1, NF], f32, kind="ExternalInput").ap()
    gbq = nc.dram_tensor("gbq", [2, DH], f32, kind="ExternalInput").ap()
    gbk = nc.dram_tensor("gbk", [2, DH], f32, kind="ExternalInput").ap()
    out = nc.dram_tensor("out", [S, DM], f32, kind="ExternalOutput").ap()

    out_t = out.rearrange("(t p) n -> p t n", p=128)       # [128, 16, 1536]
    xq_t = xq.rearrange("(t p) c -> p t c", p=128)         # [128, 16, 3]
    xk_t = xk.rearrange("(t p) c -> p t c", p=128)

    with tile.TileContext(nc) as tc, ExitStack() as ctx:
        consts = ctx.enter_context(tc.tile_pool(name="consts", bufs=1))

        ident = consts.tile([128, 128], bf, tag="ident")
        make_identity(nc, ident)

        freqs_sb = consts.tile([1, NF], f32, tag="freqs1")
        nc.sync.dma_start(out=freqs_sb, in_=freqs)
        freqs_bc = consts.tile([128, NF], f32, tag="freqsbc")
        nc.gpsimd.partition_broadcast(freqs_bc, freqs_sb)

        eps_sb = consts.tile([128, 1], f32, tag="eps")
        nc.vector.memset(eps_sb, LN_EPS)

        # gamma/beta broadcast to all partitions: gb128[p, qk, {gamma,beta}, d]
        gbq_sb = consts.tile([1, 2, DH], f32, tag="gbq")
        nc.sync.dma_start(out=gbq_sb,
                          in_=gbq.rearrange("(o a) d -> o a d", o=1))
        gbk_sb = consts.tile([1, 2, DH], f32, tag="gbk")
        nc.sync.dma_start(out=gbk_sb,
                          in_=gbk.rearrange("(o a) d -> o a d", o=1))
        gb128 = consts.tile([128, 2, 2, DH], f32, tag="gb128")
        nc.gpsimd.partition_broadcast(
            gb128[:, 0].rearrange("p b d -> p (b d)"),
            gbq_sb.rearrange("o b d -> o (b d)"))
        nc.gpsimd.partition_broadcast(
            gb128[:, 1].rearrange("p b d -> p (b d)"),
            gbk_sb.rearrange("o b d -> o (b d)"))

        xq_sb = consts.tile([128, SQ_TILES, PHYS], f32, tag="xq")
        nc.sync.dma_start(out=xq_sb, in_=xq_t)
        xk_sb = consts.tile([128, SQ_TILES, PHYS], f32, tag="xk")
        nc.sync.dma_start(out=xk_sb, in_=xk_t)

        # persistent per-head activations
        heads = ctx.enter_context(tc.tile_pool(name="heads", bufs=1))
        qT_all = heads.tile([DH, HG, S], bf, tag="qT_all")
        kT_all = heads.tile([DH, HG, S], bf, tag="kT_all")
        # v with a leading ones column per head: [sk_part, sk_tile, head, 1+96]
        v_aug = heads.tile([128, SQ_TILES, HG, 1 + DH], bf, tag="v_aug")
        nc.vector.memset(v_aug[:, :, :, 0:1], 1.0)
        # normalized y^T per head (matmul lhsT needs base partition 0)
        yN_all = heads.tile([DH, HG, S], bf, tag="yN_all")

        # ---------------- projections + LN + RoPE + transposes ----------------
        def evict_ln_rope(tensor_idx, t, ps_chunks, work, psT, dst_T):
            """LN + gamma/beta + rope on q/k psum chunks of sq-tile t, then
            per-head PE-transpose into dst_T ([96,2048] bf16 per head)."""
            xln = work.tile([128, 2 * PROJ_CHUNK], f32, tag="xln")
            xln4 = xln.rearrange("p (c h d) -> p (c h) d", c=2, d=DH)
            for c in range(2):
                ps = ps_chunks[c]
                stats = work.tile([128, 4, 6], f32, tag="stats")
                for h4 in range(4):
                    nc.vector.bn_stats(
                        out=stats[:, h4, :],
                        in_=ps.rearrange("p (h d) -> p h d", d=DH)[:, h4, :])
                mv = work.tile([128, 4, 2], f32, tag="mv")
                for h4 in range(4):
                    nc.vector.bn_aggr(out=mv[:, h4, :], in_=stats[:, h4, :])
                rstd = work.tile([128, 4], f32, tag="rstd")
                nc.scalar.activation(out=rstd, in_=mv[:, :, 1],
                                     func=AF.Sqrt, bias=eps_sb, scale=1.0)
                nc.vector.reciprocal(out=rstd, in_=rstd)
                for h4 in range(4):
                    nc.vector.tensor_scalar(
                        out=xln4[:, 4 * c + h4, :],
                        in0=ps.rearrange("p (h d) -> p h d", d=DH)[:, h4, :],
                        scalar1=mv[:, h4, 0:1], scalar2=rstd[:, h4:h4 + 1],
                        op0=ALU.subtract, op1=ALU.mult)
            # gamma/beta (identity for the given data, kept for generality)
            xln3 = xln.rearrange("p (h d) -> p h d", d=DH)
            gammab = gb128[:, tensor_idx, 0, :].rearrange(
                "p (o d) -> p o d", o=1).broadcast_to([128, HG, DH])
            betab = gb128[:, tensor_idx, 1, :].rearrange(
                "p (o d) -> p o d", o=1).broadcast_to([128, HG, DH])
            nc.vector.tensor_tensor(out=xln3, in0=xln3, in1=gammab, op=ALU.mult)
            nc.vector.tensor_tensor(out=xln3, in0=xln3, in1=betab, op=ALU.add)
            # rope angles
            x_sb = xq_sb if tensor_idx == 0 else xk_sb
            theta = work.tile([128, PHYS * NF], f32, tag="theta")
            for p in range(PHYS):
                nc.vector.tensor_scalar_mul(
                    out=theta[:, p * NF:(p + 1) * NF], in0=freqs_bc,
                    scalar1=x_sb[:, t, p:p + 1])
            # range-reduce for ACT Sin (valid domain [-pi, pi]):
            # k = round(theta/2pi) via the fp32 magic-number trick, then
            # Cody-Waite cascade theta - k*2pi, then wrap into [-pi, pi].
            MAGIC = 1.5 * 2.0 ** 23
            kmul = work.tile([128, PHYS * NF], f32, tag="kmul")
            nc.vector.tensor_scalar(out=kmul, in0=theta,
                                    scalar1=1.0 / (2 * math.pi),
                                    scalar2=MAGIC, op0=ALU.mult, op1=ALU.add)
            nc.vector.tensor_single_scalar(out=kmul, in_=kmul, scalar=MAGIC,
                                           op=ALU.subtract)
            nc.vector.cody_waite_cascade(out=theta, x=theta, k=kmul,
                                         c1=CW1, c2=CW2, c3=CW3)
            ts_ = kmul   # kmul's value is dead; reuse its slot
            tcs = work.tile([128, PHYS * NF], f32, tag="tcs")
            nc.vector.add_range_wrap(out=ts_, in_=theta, shift=0.0,
                                     bound=math.pi, period=2 * math.pi)
            nc.vector.add_range_wrap(out=tcs, in_=theta, shift=math.pi / 2,
                                     bound=math.pi, period=2 * math.pi)
            cos48 = work.tile([128, PHYS * NF], f32, tag="cos48")
            sin48 = work.tile([128, PHYS * NF], f32, tag="sin48")
            nc.scalar.activation(out=cos48, in_=tcs, func=AF.Sin,
                                 bias=0.0, scale=1.0)
            nc.scalar.activation(out=sin48, in_=ts_, func=AF.Sin,
                                 bias=0.0, scale=1.0)
            # rope: pairs are (even, odd) along each head's 96 dims
            xe = xln.rearrange("p (h d) -> p h d", d=DH)[:, :, 0::2]  # [128,8,48]
            xo = xln.rearrange("p (h d) -> p h d", d=DH)[:, :, 1::2]
            cosb = cos48.rearrange("p (o f) -> p o f", o=1).broadcast_to(
                [128, HG, PHYS * NF])
            sinb = sin48.rearrange("p (o f) -> p o f", o=1).broadcast_to(
                [128, HG, PHYS * NF])
            t1 = work.tile([128, HG, PHYS * NF], f32, tag="t1")
            t2 = work.tile([128, HG, PHYS * NF], f32, tag="t2")
            rot = work.tile([128, DV], bf, tag="rot")
            rote = rot.rearrange("p (h d) -> p h d", d=DH)[:, :, 0::2]
            roto = rot.rearrange("p (h d) -> p h d", d=DH)[:, :, 1::2]
            nc.vector.tensor_mul(out=t1, in0=xe, in1=cosb)
            nc.vector.tensor_mul(out=t2, in0=xo, in1=sinb)
            nc.vector.tensor_sub(out=rote, in0=t1, in1=t2)
            nc.vector.tensor_mul(out=t1, in0=xe, in1=sinb)
            nc.vector.tensor_mul(out=t2, in0=xo, in1=cosb)
            nc.vector.tensor_add(out=roto, in0=t1, in1=t2)
            # transpose each head's [128, 96] block; batch 4 heads per psum
            # tile so the psum->SBUF eviction is one op per 4 heads
            for c in range(2):
                tp = psT.tile([DH, 4, 128], bf, tag="tp")
                for i in range(4):
                    nc.tensor.transpose(
                        out=tp[:, i, :],
                        in_=rot[:, (4 * c + i) * DH:(4 * c + i + 1) * DH],
                        identity=ident)
                nc.scalar.copy(
                    out=dst_T[:, 4 * c:4 * c + 4, t * 128:(t + 1) * 128],
                    in_=tp)

        with ExitStack() as proj_ctx:
            xT_pool = proj_ctx.enter_context(tc.tile_pool(name="xT", bufs=2))
            w_pool = proj_ctx.enter_context(tc.tile_pool(name="w", bufs=1))
            work = proj_ctx.enter_context(tc.tile_pool(name="work", bufs=2))
            ps_pool = proj_ctx.enter_context(
                tc.tile_pool(name="ps_proj", bufs=4, space="PSUM"))
            psT_pool = proj_ctx.enter_context(
                tc.tile_pool(name="ps_tp", bufs=4, space="PSUM"))

            SH = S // 2
            for tensor_idx, (x_dram, w_dram) in enumerate(
                    [(qx, wqt), (kx, wkt), (vx, wvt)]):
                w_sb = w_pool.tile([128, K_TILES, DV], bf, tag="w")
                nc.sync.dma_start(
                    out=w_sb, in_=w_dram.rearrange("(j p) n -> p j n", p=128))
                for half in range(2):
                    xT = xT_pool.tile([128, K_TILES, SH], bf, tag="xT")
                    for j in range(K_TILES):
                        nc.sync.dma_start_transpose(
                            out=xT[:, j, :],
                            in_=x_dram[half * SH:(half + 1) * SH,
                                       j * 128:(j + 1) * 128])
                    for tl in range(SH // 128):
                        t = half * (SH // 128) + tl
                        ps_chunks = []
                        for c in range(2):
                            ps = ps_pool.tile([128, PROJ_CHUNK], f32, tag="proj")
                            for j in range(K_TILES):
                                nc.tensor.matmul(
                                    ps, lhsT=xT[:, j, tl * 128:(tl + 1) * 128],
                                    rhs=w_sb[:, j,
                                             c * PROJ_CHUNK:(c + 1) * PROJ_CHUNK],
                                    start=(j == 0), stop=(j == K_TILES - 1))
                            ps_chunks.append(ps)
                        if tensor_idx < 2:
                            evict_ln_rope(tensor_idx, t, ps_chunks, work,
                                          psT_pool,
                                          qT_all if tensor_idx == 0 else kT_all)
                        else:
                            for c in range(2):
                                nc.vector.tensor_copy(
                                    out=v_aug[:, t, 4 * c:4 * c + 4, 1:],
                                    in_=ps_chunks[c].rearrange(
                                        "p (h d) -> p h d", d=DH))

        # ---------------- attention ----------------
        with ExitStack() as att_ctx:
            e_pool = att_ctx.enter_context(tc.tile_pool(name="E", bufs=4))
            s_pool = att_ctx.enter_context(
                tc.tile_pool(name="ps_s", bufs=3, space="PSUM"))
            y_pool = att_ctx.enter_context(
                tc.tile_pool(name="ps_y", bufs=1, space="PSUM"))
            nrm = att_ctx.enter_context(tc.tile_pool(name="nrm", bufs=2))

            SH2 = S // 2
            for h in range(HG):
                for half in range(2):
                    y_ps = [y_pool.tile([1 + DH, 512], f32, tag=f"y{c}",
                                        name=f"y_ps{c}") for c in range(2)]
                    for sk in range(SQ_TILES):
                        e_tile = e_pool.tile([128, SH2], bf, tag="E")
                        kslice = kT_all[:, h, sk * 128:(sk + 1) * 128]
                        s_ps = s_pool.tile([128, 2, 512], f32, tag="S")
                        for i in range(2):
                            nc.tensor.matmul(
                                s_ps[:, i, :], lhsT=kslice,
                                rhs=qT_all[:, h, half * SH2 + i * 512:
                                           half * SH2 + (i + 1) * 512],
                                start=True, stop=True)
                        nc.scalar.activation(
                            out=e_tile,
                            in_=s_ps.rearrange("p a b -> p (a b)"),
                            func=AF.Exp, scale=SCALE)
                        for i in range(2):
                            nc.tensor.matmul(
                                y_ps[i], lhsT=v_aug[:, sk, h, :],
                                rhs=e_tile[:, i * 512:(i + 1) * 512],
                                start=(sk == 0), stop=(sk == SQ_TILES - 1))
                    yst = nrm.tile([1 + DH, SH2], bf, tag="yst")
                    for i in range(2):
                        r1 = nrm.tile([1, 512], f32, tag="r1")
                        nc.vector.reciprocal(out=r1, in_=y_ps[i][0:1, :])
                        rbc = nrm.tile([1 + DH, 512], f32, tag="rbc")
                        nc.gpsimd.partition_broadcast(rbc, r1)
                        nc.vector.tensor_tensor(
                            out=yst[:, i * 512:(i + 1) * 512],
                            in0=y_ps[i], in1=rbc, op=ALU.mult)
                    # shift rows 1..96 down to partitions 0..95 (DMA remap)
                    nc.sync.dma_start(
                        out=yN_all[:, h, half * SH2:(half + 1) * SH2],
                        in_=yst[1:, :])

        # ---------------- output projection ----------------
        with ExitStack() as op_ctx:
            wo_pool = op_ctx.enter_context(tc.tile_pool(name="wo", bufs=1))
            o_pool = op_ctx.enter_context(
                tc.tile_pool(name="ps_o", bufs=3, space="PSUM"))
            oev = op_ctx.enter_context(tc.tile_pool(name="oev", bufs=4))

            woT = [wo_pool.tile([DH, DM], bf, tag=f"wo{h}", name=f"woT{h}")
                   for h in range(HG)]
            for h in range(HG):
                nc.sync.dma_start(out=woT[h],
                                  in_=wot[h * DH:(h + 1) * DH, :])
            for t in range(SQ_TILES):
                for c3 in range(3):
                    o_ps = o_pool.tile([128, 512], f32, tag="o")
                    for h in range(HG):
                        nc.tensor.matmul(
                            o_ps, lhsT=yN_all[:, h, t * 128:(t + 1) * 128],
                            rhs=woT[h][:, c3 * 512:(c3 + 1) * 512],
                            start=(h == 0), stop=(h == HG - 1))
                    o_sb = oev.tile([128, 512], f32, tag="osb")
                    nc.scalar.copy(out=o_sb, in_=o_ps)
                    nc.sync.dma_start(out=out_t[:, t, c3 * 512:(c3 + 1) * 512],
                                      in_=o_sb)

    nc.compile()
    return nc


_PROGRAM = None


def _get_program():
    global _PROGRAM
    if _PROGRAM is None:
        _PROGRAM = build_program()
    return _PROGRAM


def make_in_maps(qx, kx, vx, x_q, x_k, Wq, Wk, Wv, Wo, q_gamma, q_beta,
                 k_gamma, k_beta):
    freqs = np.exp(np.linspace(MIN_LF, MAX_LF, NF)).astype(np.float32)
    in_maps = []
    for core in range(N_CORES):
        b, g = core // 2, core % 2
        rows = slice(g * DV, (g + 1) * DV)
        in_maps.append({
            "qx": np.ascontiguousarray(qx[b]).astype(_bf16),
            "kx": np.ascontiguousarray(kx[b]).astype(_bf16),
            "vx": np.ascontiguousarray(vx[b]).astype(_bf16),
            "wqt": np.ascontiguousarray(Wq[rows].T).astype(_bf16),
            "wkt": np.ascontiguousarray(Wk[rows].T).astype(_bf16),
            "wvt": np.ascontiguousarray(Wv[rows].T).astype(_bf16),
            "wot": np.ascontiguousarray(Wo[:, rows].T).astype(_bf16),
            "xq": np.ascontiguousarray(x_q[b]).astype(np.float32),
            "xk": np.ascontiguousarray(x_k[b]).astype(np.float32),
            "freqs": freqs[None, :],
            "gbq": np.stack([q_gamma, q_beta]).astype(np.float32),
            "gbk": np.stack([k_gamma, k_beta]).astype(np.float32),
        })
    return in_maps


LAST_EXEC_TIME_NS = None


def kernel(qx, kx, vx, x_q, x_k, Wq, Wk, Wv, Wo, q_gamma, q_beta,
           k_gamma, k_beta):
    global LAST_EXEC_TIME_NS
    import os
    _install_axon_hooks()
    from concourse.bass_utils import run_bass_kernel_spmd

    nc = _get_program()
    in_maps = make_in_maps(np.asarray(qx), np.asarray(kx), np.asarray(vx),
                           np.asarray(x_q), np.asarray(x_k), np.asarray(Wq),
                           np.asarray(Wk), np.asarray(Wv), np.asarray(Wo),
                           np.asarray(q_gamma), np.asarray(q_beta),
                           np.asarray(k_gamma), np.asarray(k_beta))
    trace = bool(int(os.environ.get("KERNEL_TRACE", "0")))
    res = run_bass_kernel_spmd(nc, in_maps, list(range(N_CORES)), trace=trace)
    LAST_EXEC_TIME_NS = res.exec_time_ns
    outv = np.empty((B, S, DM), np.float32)
    for b in range(B):
        outv[b] = res.results[2 * b]["out"] + res.results[2 * b + 1]["out"]
    return outv
